# revision 14
# baseline (speedup 1.0000x reference)
"""CBAM attention (channel + spatial) Trainium2 Bass kernel — bf16 v3.

Full inputs in, full output out. Data-parallel over batch: B=32 samples
split 4-per-core across 8 NeuronCores; params replicated.

v3 design (vs v2 at 227us). Engine budget per sample (ns, measured op
costs): DVE = maxtree 4x2.64u + final 4x2.19u + x2 2x1.08u + chain
3x2.19u + cmaxred 4x1.19u ~= 33u; ACT = avg 4x3.7u + x2 2x3.7u +
smalls ~= 24u; PE ~= 23u. Changes:
  - per-chunk x loads (4 DMAs/sample) with per-chunk pooling chained
    right behind each landing chunk (kills the 21us startup stall).
  - avgpool on ACT accum (scale=1/HWF folded); maxpool per-chunk DVE
    tt-max tree (2x bf16 mode; tensor_scalar-accum / stt / native TTR
    all measured slower or broken in walrus).
  - finals of sample b-1 issued between mlp(b) and x2(b) on DVE,
    filling the s_t(b) MLP wait.
  - csum accumulates into 2-bank PSUM waves DMA'd straight into the
    f32 conv_in (no 1-partition ACT copies).
  - MLP fused: avg/max as 2-col moving per chunk; W2 stationary gets a
    bias row (hs1[64]=1.0) so sigmoid needs no per-chunk bias; one
    [128,4] sigmoid.
  - conv in f32 end to end (s6f f32 stationary, conv_in f32).
  - per-chunk final multiply -> immediate per-chunk store.

Walrus constraints honored: one sync wait per instruction (post-pass
split), no raw-ISA DVE/Pool ops (TTR/mask_reduce/partition_broadcast
all hit "ISA wrong length"), no partition-base mismatch between SBUF
operands.
"""
import numpy as np
from contextlib import ExitStack

import concourse.bass as bass
import concourse.mybir as mybir
from concourse.tile import TileContext
from concourse.bass_utils import run_bass_kernel_spmd

F32 = mybir.dt.float32
BF16 = mybir.dt.bfloat16
ALU = mybir.AluOpType
ACTF = mybir.ActivationFunctionType
AX = mybir.AxisListType

B, C, H, W = 32, 512, 64, 64
NCORES = 8
BPC = B // NCORES          # samples per core
HWF = H * W                # 4096
NCH = C // 128             # 4 channel chunks of 128
Cr = C // 8                # 64
EPS = 1e-12

_cache = {}


def _split_multi_waits(nc):
    import bass_rust
    fn = nc.m.functions[0]
    n_split = 0
    uid = 0
    for bb in list(fn.blocks):
        insts = bb.instructions
        out = []
        changed = False
        for ins in insts:
            si = ins.sync_info
            waits = list(si.on_wait) if si and si.on_wait else []
            if len(waits) > 1:
                changed = True
                for w in waits[:-1]:
                    nop = bass_rust.InstNoOp(name=f"wsplit_{uid}")
                    uid += 1
                    nop.engine = ins.engine
                    nop.sync_info = bass_rust.SyncInfo(on_wait=[w], on_update=[])
                    nc.register_instruction(nop, overwrite=True)
                    out.append(nop)
                    n_split += 1
                ins.sync_info = bass_rust.SyncInfo(
                    on_wait=waits[-1:], on_update=list(si.on_update or []))
            out.append(ins)
        if changed:
            bb.instructions = out
    return n_split


def _build_nc():
    nc = bass.Bass("TRN2", debug=False)

    x_ext = nc.declare_dram_parameter("x", [BPC, C, H, W], BF16, isOutput=False)
    w1t_ext = nc.declare_dram_parameter("w1t", [128, NCH, Cr], F32, isOutput=False)
    b1_ext = nc.declare_dram_parameter("b1", [Cr, 1], F32, isOutput=False)
    w2te_ext = nc.declare_dram_parameter("w2te", [Cr + 1, NCH, 128], F32,
                                         isOutput=False)
    s6f_ext = nc.declare_dram_parameter("s6f", [H, 6, H], F32, isOutput=False)
    s6cm_ext = nc.declare_dram_parameter("s6cm", [32, 6, H], F32, isOutput=False)
    b3r_ext = nc.declare_dram_parameter("b3r", [H, 1], F32, isOutput=False)
    ones16_ext = nc.declare_dram_parameter("ones16", [128, 1], BF16, isOutput=False)
    id16_ext = nc.declare_dram_parameter("id16", [128, 128], BF16, isOutput=False)
    idf_ext = nc.declare_dram_parameter("idf", [128, 128], F32, isOutput=False)
    out_ext = nc.declare_dram_parameter("out", [BPC, C, H, W], BF16, isOutput=True)
    sig_scr = nc.dram_tensor("sig_scratch", [BPC, H, W], BF16)

    # DRAM views: [128 part, chunk, hw]
    x_v = [x_ext[b].rearrange("c h w -> c (h w)").rearrange("(g p) f -> p g f", p=128)
           for b in range(BPC)]
    o_v = [out_ext[b].rearrange("c h w -> c (h w)").rearrange("(g p) f -> p g f", p=128)
           for b in range(BPC)]

    with TileContext(nc) as tc, ExitStack() as ctx:
        const = ctx.enter_context(tc.tile_pool(name="const", bufs=1))
        xpool = ctx.enter_context(tc.tile_pool(name="x", bufs=12))
        t1p = ctx.enter_context(tc.tile_pool(name="t1", bufs=2))
        scr = ctx.enter_context(tc.tile_pool(name="scr", bufs=1))
        sigp = ctx.enter_context(tc.tile_pool(name="sig", bufs=2))
        stats = ctx.enter_context(tc.tile_pool(name="stats", bufs=2))
        convp = ctx.enter_context(tc.tile_pool(name="conv", bufs=2))
        pstp = ctx.enter_context(tc.tile_pool(name="pstp", bufs=2, space="PSUM"))
        pscs = ctx.enter_context(tc.tile_pool(name="pscs", bufs=2, space="PSUM"))
        psmlp = ctx.enter_context(tc.tile_pool(name="psmlp", bufs=1, space="PSUM"))
        psc2 = ctx.enter_context(tc.tile_pool(name="psc2", bufs=1, space="PSUM"))

        # ---------------- load params ----------------
        w1t = const.tile([128, NCH, Cr], F32)
        nc.scalar.dma_start(out=w1t[:], in_=w1t_ext[:])
        b1t = const.tile([Cr, 1], F32)
        nc.scalar.dma_start(out=b1t[:], in_=b1_ext[:])
        w2te = const.tile([Cr + 1, NCH, 128], F32)
        nc.scalar.dma_start(out=w2te[:], in_=w2te_ext[:])
        s6f = const.tile([H, 6, H], F32)
        nc.scalar.dma_start(out=s6f[:], in_=s6f_ext[:])
        s6cm = const.tile([32, 6, H], F32)
        nc.scalar.dma_start(out=s6cm[:], in_=s6cm_ext[:])
        b3rep = const.tile([H, 1], F32)
        nc.scalar.dma_start(out=b3rep[:], in_=b3r_ext[:])
        ones16 = const.tile([128, 1], BF16)
        nc.scalar.dma_start(out=ones16[:], in_=ones16_ext[:])
        ident16 = const.tile([128, 128], BF16)
        nc.scalar.dma_start(out=ident16[:], in_=id16_ext[:])
        identf = const.tile([128, 128], F32)
        nc.scalar.dma_start(out=identf[:], in_=idf_ext[:])

        # shared scratch (write-only / same-engine serial reuse)
        dummy = scr.tile([128, HWF], BF16)        # ACT avg-pool main out
        maxA = scr.tile([128, HWF // 2], BF16)    # maxpool tree ping
        maxB = scr.tile([128, HWF // 4], BF16)    # maxpool tree pong
        chainscr = scr.tile([128, HWF], BF16)     # chain max t23

        st = {}

        def ph_load(b):
            xc = []
            for c in range(NCH):
                t = xpool.tile([128, HWF], BF16, tag="x")
                eng = nc.sync
                if b == 0:
                    # startup-critical: halve for faster first landing
                    eng.dma_start(out=t[:, 0:HWF // 2],
                                  in_=x_v[b][:, c, 0:HWF // 2])
                    eng.dma_start(out=t[:, HWF // 2:HWF],
                                  in_=x_v[b][:, c, HWF // 2:HWF])
                else:
                    eng.dma_start(out=t[:], in_=x_v[b][:, c, :])
                xc.append(t)
            st[b] = {"xc": xc}

        def ph_pool_chunk(b, c):
            z = st[b]
            if c == 0:
                stt8 = stats.tile([128, 2 * NCH], F32, tag="st8")
                z["stt"] = stt8
            stt = z["stt"]
            h2, h4, h8 = HWF // 2, HWF // 4, HWF // 8
            xcc = z["xc"][c]
            # ACT: mean accumulate (scale folds the 1/HW)
            nc.scalar.activation(out=dummy[:], in_=xcc[:], func=ACTF.Copy,
                                 scale=1.0 / HWF,
                                 accum_out=stt[:, 2 * c:2 * c + 1])
            # DVE: max tree
            nc.vector.tensor_tensor(maxA[:, 0:h2], xcc[:, 0:h2],
                                    xcc[:, h2:HWF], ALU.max)
            nc.vector.tensor_tensor(maxB[:, 0:h4], maxA[:, 0:h4],
                                    maxA[:, h4:h2], ALU.max)
            nc.vector.tensor_tensor(maxA[:, 0:h8], maxB[:, 0:h8],
                                    maxB[:, h8:h4], ALU.max)
            nc.vector.tensor_reduce(out=stt[:, 2 * c + 1:2 * c + 2],
                                    in_=maxA[:, 0:h8], axis=AX.X, op=ALU.max)

        def ph_mlp(b):
            z = st[b]
            stt = z["stt"]
            mlpp = psmlp.tile([128, 6], F32, tag="mlp")
            h_ps = mlpp[0:Cr, 0:2]
            for c in range(NCH):
                nc.tensor.matmul(h_ps, w1t[:, c, :], stt[:, 2 * c:2 * c + 2],
                                 start=(c == 0), stop=(c == NCH - 1))
            h_sb = stats.tile([Cr, 2], F32, tag="hsb")
            nc.scalar.activation(out=h_sb[:], in_=h_ps, func=ACTF.Relu,
                                 bias=b1t[:])
            hs1 = stats.tile([Cr + 1, 1], F32, tag="hs1")
            nc.scalar.activation(out=hs1[0:Cr, :], in_=h_sb[:, 0:1],
                                 func=ACTF.Relu, bias=h_sb[:, 1:2])
            nc.gpsimd.memset(hs1[Cr:Cr + 1, :], 1.0)
            a_ps = mlpp[:, 2:6]
            for c in range(NCH):
                nc.tensor.matmul(a_ps[:, c:c + 1], w2te[:, c, :], hs1[:],
                                 start=True, stop=True)
            s_t = stats.tile([128, NCH], F32, tag="sf")
            nc.scalar.activation(out=s_t[:], in_=a_ps, func=ACTF.Sigmoid)
            z["s_t"] = s_t

        def ph_x2(b):
            z = st[b]
            xc, s_t = z["xc"], z["s_t"]
            # DVE chunks 0,1 (ts 4x); ACT chunks 2,3
            for c in (0, 1):
                nc.vector.tensor_scalar(xc[c][:], xc[c][:], s_t[:, c:c + 1],
                                        None, ALU.mult)
            for c in (2, 3):
                nc.scalar.activation(out=xc[c][:], in_=xc[c][:], func=ACTF.Copy,
                                     scale=s_t[:, c:c + 1])

        def ph_chain(b):
            z = st[b]
            xc = z["xc"]
            t1 = t1p.tile([128, HWF], BF16, tag="t1")
            nc.vector.tensor_tensor(t1[:], xc[0][:], xc[1][:], ALU.max)
            nc.vector.tensor_tensor(chainscr[:], xc[2][:], xc[3][:], ALU.max)
            for g in range(4):
                lo, hi = 1024 * g, 1024 * (g + 1)
                nc.vector.tensor_tensor(t1[:, lo:hi], t1[:, lo:hi],
                                        chainscr[:, lo:hi], ALU.max)
            z["t1"] = t1

        def ph_spatial(b):
            # csum waves interleaved with cmax transpose groups on PE:
            # wave0, wave1, [tp g0], wave2, [tp g1], wave3, [tp g2], [tp g3]
            z = st[b]
            xc, t1 = z["xc"], z["t1"]
            cia = convp.tile([H, W + 2], F32, tag="cia")
            cim = convp.tile([32, 2, W + 2], F32, tag="cim")
            nc.gpsimd.memset(cia[:, 0:1], 0.0)
            nc.gpsimd.memset(cia[:, W + 1:W + 2], 0.0)
            nc.gpsimd.memset(cim[:, :, 0:1], 0.0)
            nc.gpsimd.memset(cim[:, :, W + 1:W + 2], 0.0)
            cmaxT = stats.tile([128, 32], F32, tag="cmaxT")

            def csum_wave(w):
                ps = pscs.tile([1, 2, 512], F32, tag="cs")
                for c in range(NCH):
                    for j in range(2):
                        q = 2 * w + j
                        nc.tensor.matmul(ps[0:1, j, :], ones16[:],
                                         xc[c][:, 512 * q:512 * (q + 1)],
                                         start=(c == 0), stop=(c == NCH - 1))
                crow = convp.tile([1, 1024], F32, tag="crow")
                nc.scalar.copy(crow[:], ps[:].rearrange("p j w -> p (j w)"))
                nc.scalar.dma_start(
                    out=cia[16 * w:16 * (w + 1), 1:W + 1],
                    in_=crow[:].rearrange("p (h ww) -> p h ww", ww=W))

            def tp_group(g):
                ps = pstp.tile([128, 8, 128], BF16, tag="tp")
                for mm in range(8):
                    m = 8 * g + mm
                    nc.tensor.transpose(ps[:, mm, :], t1[:, 128 * m:128 * (m + 1)],
                                        ident16[:])
                nc.vector.tensor_reduce(out=cmaxT[:, 8 * g:8 * (g + 1)], in_=ps[:],
                                        axis=AX.X, op=ALU.max)

            csum_wave(0)
            csum_wave(1)
            tp_group(0)
            csum_wave(2)
            tp_group(1)
            csum_wave(3)
            tp_group(2)
            tp_group(3)

            cvt1 = psc2.tile([64, 192], F32, tag="cvt")
            t2ps = cvt1[0:32, 0:128]
            nc.tensor.transpose(t2ps, cmaxT[:], identf[:])
            nc.scalar.copy(cim[:, :, 1:W + 1],
                           t2ps.rearrange("q (r w) -> q r w", r=2))
            z["cia"], z["cim"] = cia, cim

        def ph_conv_mm(b):
            z = st[b]
            cia, cim = z["cia"], z["cim"]
            cvt2 = psc2.tile([64, 192], F32, tag="cvt")
            yps = cvt2[0:H, 128:128 + W]
            j = 0
            for dw in range(3):
                nc.tensor.matmul(yps, s6f[:, dw, :], cia[:, dw:dw + W],
                                 start=(j == 0), stop=False)
                j += 1
            for r in range(2):
                for dw in range(3):
                    nc.tensor.matmul(yps, s6cm[:, 3 * r + dw, :],
                                     cim[:, r, dw:dw + W],
                                     start=False, stop=(j == 8))
                    j += 1
            z["yps"] = yps

        def ph_conv_sig(b):
            z = st[b]
            yps = z["yps"]
            y16 = convp.tile([H, W], BF16, tag="y16")
            nc.scalar.activation(out=y16[:], in_=yps, func=ACTF.Sigmoid,
                                 bias=b3rep[:])
            nc.scalar.dma_start(out=sig_scr[b], in_=y16[:])
            flat = sig_scr[b].rearrange("h w -> (h w)")
            bcast_ap = bass.AP(tensor=flat.tensor, offset=flat.offset,
                               ap=[[0, 128]] + list(flat.ap))
            sigB = sigp.tile([128, HWF], BF16, tag="sigB")
            nc.scalar.dma_start(out=sigB[:], in_=bcast_ap)
            z["sigB"] = sigB

        def ph_final(b, cs):
            z = st[b]
            xc, sigB = z["xc"], z["sigB"]
            for c in cs:
                nc.vector.tensor_tensor(xc[c][:], xc[c][:], sigB[:], ALU.mult)
                nc.sync.dma_start(out=o_v[b][:, c, :], in_=xc[c][:])
            if cs[-1] == NCH - 1:
                del st[b]

        # pipeline: conv_mm(i-2) | load(i) || pool+mlp(i-1) with
        # conv_sig(i-2) after the first avg chunk | final(i-2) |
        # x2+chain+spatial(i-1)
        for i in range(BPC + 2):
            if 2 <= i:
                ph_conv_mm(i - 2)
            if i < BPC:
                ph_load(i)
            if 1 <= i <= BPC:
                ph_pool_chunk(i - 1, 0)
            if 2 <= i:
                ph_conv_sig(i - 2)
            if 1 <= i <= BPC:
                b = i - 1
                for c in (1, 2, 3):
                    ph_pool_chunk(b, c)
                ph_mlp(b)
            if 2 <= i:
                ph_final(i - 2, (0, 1))
            if 1 <= i <= BPC:
                b = i - 1
                ph_x2(b)
                ph_chain(b)
                ph_spatial(b)
            if 2 <= i:
                ph_final(i - 2, (2, 3))

    _split_multi_waits(nc)
    return nc


def _get_nc():
    if "nc" not in _cache:
        _cache["nc"] = _build_nc()
    return _cache["nc"]


def _sn_inv(w, u):
    """1/spectral-norm estimate, mirroring reference._sn (f32 numpy)."""
    w = np.asarray(w, np.float32)
    u = np.asarray(u, np.float32)
    Wm = w.reshape(w.shape[0], -1)
    v = u @ Wm
    v = v / max(np.linalg.norm(v), EPS)
    u2 = v @ Wm.T
    u2 = u2 / max(np.linalg.norm(u2), EPS)
    sv = float(np.squeeze(v @ Wm.T @ u2.T))
    return 1.0 / sv


def _prep_in_maps(inputs):
    import ml_dtypes
    bf16 = ml_dtypes.bfloat16
    f = lambda a: np.ascontiguousarray(np.asarray(a, dtype=np.float32))
    x16 = np.ascontiguousarray(
        np.asarray(inputs["x"], dtype=np.float32).astype(bf16))

    W1 = f(inputs["w1"]).reshape(Cr, C) * _sn_inv(inputs["w1"], inputs["u1"])
    W2 = f(inputs["w2"]).reshape(C, Cr) * _sn_inv(inputs["w2"], inputs["u2"])
    taps = (f(inputs["w3"]).reshape(2, 3, 3)
            * _sn_inv(inputs["w3"], inputs["u3"])).astype(np.float32)
    taps = taps.copy()
    taps[0] /= C                               # ca is channel MEAN

    # banded conv stationaries: s6[hin, 3c+dw, hout] = taps[c, hin-hout+1, dw]
    s6 = np.zeros((H, 6, H), np.float32)
    for c in range(2):
        for dw in range(3):
            for hout in range(H):
                for kh in range(3):
                    hin = hout + kh - 1
                    if 0 <= hin < H:
                        s6[hin, 3 * c + dw, hout] = taps[c, kh, dw]
    # cm stationaries for [32,2,66] layout: hin = 2p + r
    s6cm = np.zeros((32, 6, H), np.float32)
    for r in range(2):
        for dw in range(3):
            for p in range(32):
                hin = 2 * p + r
                for hout in range(H):
                    kh = hin - hout + 1
                    if 0 <= kh < 3:
                        s6cm[p, 3 * r + dw, hout] = taps[1, kh, dw]

    # W2^T extended with a 2*b2 bias row (moving hs1[Cr]=1.0)
    w2t = np.ascontiguousarray(W2.T.reshape(Cr, NCH, 128)).astype(np.float32)
    b2row = (2.0 * f(inputs["b2"])).reshape(1, NCH, 128).astype(np.float32)
    w2te = np.ascontiguousarray(np.concatenate([w2t, b2row], axis=0))

    common = {
        "w1t": np.ascontiguousarray(
            W1.T.reshape(NCH, 128, Cr).transpose(1, 0, 2)).astype(np.float32),
        "b1": f(inputs["b1"]).reshape(Cr, 1),
        "w2te": w2te,
        "s6f": s6,
        "s6cm": s6cm,
        "b3r": np.full((H, 1), float(np.asarray(inputs["b3"]).reshape(-1)[0]),
                       np.float32),
        "ones16": np.ones((128, 1), bf16),
        "id16": np.eye(128, dtype=bf16),
        "idf": np.eye(128, dtype=np.float32),
    }
    return [dict(common, x=np.ascontiguousarray(x16[k * BPC:(k + 1) * BPC]))
            for k in range(NCORES)]


def run(inputs, trace=False, **kw):
    nc = _get_nc()
    in_maps = _prep_in_maps(inputs)
    res = run_bass_kernel_spmd(nc, in_maps, list(range(NCORES)), trace=trace, **kw)
    out = np.concatenate(
        [np.asarray(res.results[k]["out"]).astype(np.float32)
         for k in range(NCORES)], axis=0)
    return out, res


def kernel(**inputs) -> np.ndarray:
    out, _ = run(inputs)
    return out


# revision 15
# speedup vs baseline: 1.0064x; 1.0064x over previous
"""CBAM attention (channel + spatial) Trainium2 Bass kernel — bf16 v3.

Full inputs in, full output out. Data-parallel over batch: B=32 samples
split 4-per-core across 8 NeuronCores; params replicated.

v3 design (vs v2 at 227us). Engine budget per sample (ns, measured op
costs): DVE = maxtree 4x2.64u + final 4x2.19u + x2 2x1.08u + chain
3x2.19u + cmaxred 4x1.19u ~= 33u; ACT = avg 4x3.7u + x2 2x3.7u +
smalls ~= 24u; PE ~= 23u. Changes:
  - per-chunk x loads (4 DMAs/sample) with per-chunk pooling chained
    right behind each landing chunk (kills the 21us startup stall).
  - avgpool on ACT accum (scale=1/HWF folded); maxpool per-chunk DVE
    tt-max tree (2x bf16 mode; tensor_scalar-accum / stt / native TTR
    all measured slower or broken in walrus).
  - finals of sample b-1 issued between mlp(b) and x2(b) on DVE,
    filling the s_t(b) MLP wait.
  - csum accumulates into 2-bank PSUM waves DMA'd straight into the
    f32 conv_in (no 1-partition ACT copies).
  - MLP fused: avg/max as 2-col moving per chunk; W2 stationary gets a
    bias row (hs1[64]=1.0) so sigmoid needs no per-chunk bias; one
    [128,4] sigmoid.
  - conv in f32 end to end (s6f f32 stationary, conv_in f32).
  - per-chunk final multiply -> immediate per-chunk store.

Walrus constraints honored: one sync wait per instruction (post-pass
split), no raw-ISA DVE/Pool ops (TTR/mask_reduce/partition_broadcast
all hit "ISA wrong length"), no partition-base mismatch between SBUF
operands.
"""
import numpy as np
from contextlib import ExitStack

import concourse.bass as bass
import concourse.mybir as mybir
from concourse.tile import TileContext
from concourse.bass_utils import run_bass_kernel_spmd

F32 = mybir.dt.float32
BF16 = mybir.dt.bfloat16
ALU = mybir.AluOpType
ACTF = mybir.ActivationFunctionType
AX = mybir.AxisListType

B, C, H, W = 32, 512, 64, 64
NCORES = 8
BPC = B // NCORES          # samples per core
HWF = H * W                # 4096
NCH = C // 128             # 4 channel chunks of 128
Cr = C // 8                # 64
EPS = 1e-12

_cache = {}


def _split_multi_waits(nc):
    import bass_rust
    fn = nc.m.functions[0]
    n_split = 0
    uid = 0
    for bb in list(fn.blocks):
        insts = bb.instructions
        out = []
        changed = False
        for ins in insts:
            si = ins.sync_info
            waits = list(si.on_wait) if si and si.on_wait else []
            if len(waits) > 1:
                changed = True
                for w in waits[:-1]:
                    nop = bass_rust.InstNoOp(name=f"wsplit_{uid}")
                    uid += 1
                    nop.engine = ins.engine
                    nop.sync_info = bass_rust.SyncInfo(on_wait=[w], on_update=[])
                    nc.register_instruction(nop, overwrite=True)
                    out.append(nop)
                    n_split += 1
                ins.sync_info = bass_rust.SyncInfo(
                    on_wait=waits[-1:], on_update=list(si.on_update or []))
            out.append(ins)
        if changed:
            bb.instructions = out
    return n_split


def _build_nc():
    nc = bass.Bass("TRN2", debug=False)

    x_ext = nc.declare_dram_parameter("x", [BPC, C, H, W], BF16, isOutput=False)
    w1t_ext = nc.declare_dram_parameter("w1t", [128, NCH, Cr], F32, isOutput=False)
    b1_ext = nc.declare_dram_parameter("b1", [Cr, 1], F32, isOutput=False)
    w2te_ext = nc.declare_dram_parameter("w2te", [Cr + 1, NCH, 128], F32,
                                         isOutput=False)
    s6f_ext = nc.declare_dram_parameter("s6f", [H, 6, H], F32, isOutput=False)
    s6cm_ext = nc.declare_dram_parameter("s6cm", [32, 6, H], F32, isOutput=False)
    b3r_ext = nc.declare_dram_parameter("b3r", [H, 1], F32, isOutput=False)
    ones16_ext = nc.declare_dram_parameter("ones16", [128, 1], BF16, isOutput=False)
    id16_ext = nc.declare_dram_parameter("id16", [128, 128], BF16, isOutput=False)
    idf_ext = nc.declare_dram_parameter("idf", [128, 128], F32, isOutput=False)
    out_ext = nc.declare_dram_parameter("out", [BPC, C, H, W], BF16, isOutput=True)
    sig_scr = nc.dram_tensor("sig_scratch", [BPC, H, W], BF16)

    # DRAM views: [128 part, chunk, hw]
    x_v = [x_ext[b].rearrange("c h w -> c (h w)").rearrange("(g p) f -> p g f", p=128)
           for b in range(BPC)]
    o_v = [out_ext[b].rearrange("c h w -> c (h w)").rearrange("(g p) f -> p g f", p=128)
           for b in range(BPC)]

    with TileContext(nc) as tc, ExitStack() as ctx:
        const = ctx.enter_context(tc.tile_pool(name="const", bufs=1))
        xpool = ctx.enter_context(tc.tile_pool(name="x", bufs=12))
        t1p = ctx.enter_context(tc.tile_pool(name="t1", bufs=2))
        scr = ctx.enter_context(tc.tile_pool(name="scr", bufs=1))
        sigp = ctx.enter_context(tc.tile_pool(name="sig", bufs=2))
        stats = ctx.enter_context(tc.tile_pool(name="stats", bufs=2))
        convp = ctx.enter_context(tc.tile_pool(name="conv", bufs=2))
        pstp = ctx.enter_context(tc.tile_pool(name="pstp", bufs=2, space="PSUM"))
        pscs = ctx.enter_context(tc.tile_pool(name="pscs", bufs=1, space="PSUM"))
        psmlp = ctx.enter_context(tc.tile_pool(name="psmlp", bufs=1, space="PSUM"))
        psc2 = ctx.enter_context(tc.tile_pool(name="psc2", bufs=1, space="PSUM"))

        # ---------------- load params ----------------
        w1t = const.tile([128, NCH, Cr], F32)
        nc.scalar.dma_start(out=w1t[:], in_=w1t_ext[:])
        b1t = const.tile([Cr, 1], F32)
        nc.scalar.dma_start(out=b1t[:], in_=b1_ext[:])
        w2te = const.tile([Cr + 1, NCH, 128], F32)
        nc.scalar.dma_start(out=w2te[:], in_=w2te_ext[:])
        s6f = const.tile([H, 6, H], F32)
        nc.scalar.dma_start(out=s6f[:], in_=s6f_ext[:])
        s6cm = const.tile([32, 6, H], F32)
        nc.scalar.dma_start(out=s6cm[:], in_=s6cm_ext[:])
        b3rep = const.tile([H, 1], F32)
        nc.scalar.dma_start(out=b3rep[:], in_=b3r_ext[:])
        ones16 = const.tile([128, 1], BF16)
        nc.scalar.dma_start(out=ones16[:], in_=ones16_ext[:])
        ident16 = const.tile([128, 128], BF16)
        nc.scalar.dma_start(out=ident16[:], in_=id16_ext[:])
        identf = const.tile([128, 128], F32)
        nc.scalar.dma_start(out=identf[:], in_=idf_ext[:])

        # shared scratch (write-only / same-engine serial reuse)
        dummy = scr.tile([128, HWF], BF16)        # ACT avg-pool main out
        maxA = scr.tile([128, HWF // 2], BF16)    # maxpool tree ping
        maxB = scr.tile([128, HWF // 4], BF16)    # maxpool tree pong
        chainscr = scr.tile([128, HWF], BF16)     # chain max t23

        st = {}

        def ph_load(b):
            xc = []
            for c in range(NCH):
                t = xpool.tile([128, HWF], BF16, tag="x")
                eng = nc.sync
                if b == 0:
                    # startup-critical: halve for faster first landing
                    eng.dma_start(out=t[:, 0:HWF // 2],
                                  in_=x_v[b][:, c, 0:HWF // 2])
                    eng.dma_start(out=t[:, HWF // 2:HWF],
                                  in_=x_v[b][:, c, HWF // 2:HWF])
                else:
                    eng.dma_start(out=t[:], in_=x_v[b][:, c, :])
                xc.append(t)
            st[b] = {"xc": xc}

        def ph_pool_chunk(b, c):
            z = st[b]
            if c == 0:
                stt8 = stats.tile([128, 2 * NCH], F32, tag="st8")
                z["stt"] = stt8
            stt = z["stt"]
            h2, h4, h8 = HWF // 2, HWF // 4, HWF // 8
            xcc = z["xc"][c]
            # ACT: mean accumulate (scale folds the 1/HW)
            nc.scalar.activation(out=dummy[:], in_=xcc[:], func=ACTF.Copy,
                                 scale=1.0 / HWF,
                                 accum_out=stt[:, 2 * c:2 * c + 1])
            # DVE: max tree
            nc.vector.tensor_tensor(maxA[:, 0:h2], xcc[:, 0:h2],
                                    xcc[:, h2:HWF], ALU.max)
            nc.vector.tensor_tensor(maxB[:, 0:h4], maxA[:, 0:h4],
                                    maxA[:, h4:h2], ALU.max)
            nc.vector.tensor_tensor(maxA[:, 0:h8], maxB[:, 0:h8],
                                    maxB[:, h8:h4], ALU.max)
            nc.vector.tensor_reduce(out=stt[:, 2 * c + 1:2 * c + 2],
                                    in_=maxA[:, 0:h8], axis=AX.X, op=ALU.max)

        def ph_mlp(b):
            z = st[b]
            stt = z["stt"]
            mlpp = psmlp.tile([128, 6], F32, tag="mlp")
            h_ps = mlpp[0:Cr, 0:2]
            for c in range(NCH):
                nc.tensor.matmul(h_ps, w1t[:, c, :], stt[:, 2 * c:2 * c + 2],
                                 start=(c == 0), stop=(c == NCH - 1))
            h_sb = stats.tile([Cr, 2], F32, tag="hsb")
            nc.scalar.activation(out=h_sb[:], in_=h_ps, func=ACTF.Relu,
                                 bias=b1t[:])
            hs1 = stats.tile([Cr + 1, 1], F32, tag="hs1")
            nc.scalar.activation(out=hs1[0:Cr, :], in_=h_sb[:, 0:1],
                                 func=ACTF.Relu, bias=h_sb[:, 1:2])
            nc.gpsimd.memset(hs1[Cr:Cr + 1, :], 1.0)
            a_ps = mlpp[:, 2:6]
            for c in range(NCH):
                nc.tensor.matmul(a_ps[:, c:c + 1], w2te[:, c, :], hs1[:],
                                 start=True, stop=True)
            s_t = stats.tile([128, NCH], F32, tag="sf")
            nc.scalar.activation(out=s_t[:], in_=a_ps, func=ACTF.Sigmoid)
            z["s_t"] = s_t

        def ph_x2(b):
            z = st[b]
            xc, s_t = z["xc"], z["s_t"]
            # DVE chunks 0,1 (ts 4x); ACT chunks 2,3. Last sample all-DVE
            # (shorter tail critical path).
            act_cs = () if b == BPC - 1 else (2, 3)
            for c in range(NCH):
                if c in act_cs:
                    nc.scalar.activation(out=xc[c][:], in_=xc[c][:],
                                         func=ACTF.Copy, scale=s_t[:, c:c + 1])
                else:
                    nc.vector.tensor_scalar(xc[c][:], xc[c][:], s_t[:, c:c + 1],
                                            None, ALU.mult)

        def ph_chain(b):
            z = st[b]
            xc = z["xc"]
            t1 = t1p.tile([128, HWF], BF16, tag="t1")
            nc.vector.tensor_tensor(t1[:], xc[0][:], xc[1][:], ALU.max)
            nc.vector.tensor_tensor(chainscr[:], xc[2][:], xc[3][:], ALU.max)
            for g in range(4):
                lo, hi = 1024 * g, 1024 * (g + 1)
                nc.vector.tensor_tensor(t1[:, lo:hi], t1[:, lo:hi],
                                        chainscr[:, lo:hi], ALU.max)
            z["t1"] = t1

        def ph_spatial(b):
            # csum waves interleaved with cmax transpose groups on PE:
            # wave0, wave1, [tp g0], wave2, [tp g1], wave3, [tp g2], [tp g3]
            z = st[b]
            xc, t1 = z["xc"], z["t1"]
            cia = convp.tile([H, W + 2], F32, tag="cia")
            cim = convp.tile([32, 2, W + 2], F32, tag="cim")
            nc.gpsimd.memset(cia[:, 0:1], 0.0)
            nc.gpsimd.memset(cia[:, W + 1:W + 2], 0.0)
            nc.gpsimd.memset(cim[:, :, 0:1], 0.0)
            nc.gpsimd.memset(cim[:, :, W + 1:W + 2], 0.0)
            cmaxT = stats.tile([128, 32], F32, tag="cmaxT")

            def csum_wave(w):
                ps = pscs.tile([1, 4, 512], F32, tag="cs")
                for c in range(NCH):
                    for j in range(4):
                        q = 4 * w + j
                        nc.tensor.matmul(ps[0:1, j, :], ones16[:],
                                         xc[c][:, 512 * q:512 * (q + 1)],
                                         start=(c == 0), stop=(c == NCH - 1))
                crow = convp.tile([1, 2048], F32, tag="crow")
                nc.scalar.copy(crow[:], ps[:].rearrange("p j w -> p (j w)"))
                nc.scalar.dma_start(
                    out=cia[32 * w:32 * (w + 1), 1:W + 1],
                    in_=crow[:].rearrange("p (h ww) -> p h ww", ww=W))

            def tp_group(g):
                ps = pstp.tile([128, 8, 128], BF16, tag="tp")
                for mm in range(8):
                    m = 8 * g + mm
                    nc.tensor.transpose(ps[:, mm, :], t1[:, 128 * m:128 * (m + 1)],
                                        ident16[:])
                nc.vector.tensor_reduce(out=cmaxT[:, 8 * g:8 * (g + 1)], in_=ps[:],
                                        axis=AX.X, op=ALU.max)

            csum_wave(0)
            tp_group(0)
            csum_wave(1)
            tp_group(1)
            tp_group(2)
            tp_group(3)

            cvt1 = psc2.tile([64, 192], F32, tag="cvt")
            t2ps = cvt1[0:32, 0:128]
            nc.tensor.transpose(t2ps, cmaxT[:], identf[:])
            nc.scalar.copy(cim[:, :, 1:W + 1],
                           t2ps.rearrange("q (r w) -> q r w", r=2))
            z["cia"], z["cim"] = cia, cim

        def ph_conv_mm(b):
            z = st[b]
            cia, cim = z["cia"], z["cim"]
            cvt2 = psc2.tile([64, 192], F32, tag="cvt")
            yps = cvt2[0:H, 128:128 + W]
            j = 0
            for dw in range(3):
                nc.tensor.matmul(yps, s6f[:, dw, :], cia[:, dw:dw + W],
                                 start=(j == 0), stop=False)
                j += 1
            for r in range(2):
                for dw in range(3):
                    nc.tensor.matmul(yps, s6cm[:, 3 * r + dw, :],
                                     cim[:, r, dw:dw + W],
                                     start=False, stop=(j == 8))
                    j += 1
            z["yps"] = yps

        def ph_conv_sig(b):
            z = st[b]
            yps = z["yps"]
            y16 = convp.tile([H, W], BF16, tag="y16")
            nc.scalar.activation(out=y16[:], in_=yps, func=ACTF.Sigmoid,
                                 bias=b3rep[:])
            nc.scalar.dma_start(out=sig_scr[b], in_=y16[:])
            flat = sig_scr[b].rearrange("h w -> (h w)")
            bcast_ap = bass.AP(tensor=flat.tensor, offset=flat.offset,
                               ap=[[0, 128]] + list(flat.ap))
            sigB = sigp.tile([128, HWF], BF16, tag="sigB")
            nc.scalar.dma_start(out=sigB[:], in_=bcast_ap)
            z["sigB"] = sigB

        def ph_final(b, cs):
            z = st[b]
            xc, sigB = z["xc"], z["sigB"]
            for c in cs:
                nc.vector.tensor_tensor(xc[c][:], xc[c][:], sigB[:], ALU.mult)
                nc.sync.dma_start(out=o_v[b][:, c, :], in_=xc[c][:])
            if cs[-1] == NCH - 1:
                del st[b]

        # pipeline: conv_mm(i-2) | load(i) || pool+mlp(i-1) with
        # conv_sig(i-2) after the first avg chunk | final(i-2) |
        # x2+chain+spatial(i-1)
        for i in range(BPC + 2):
            if 2 <= i:
                ph_conv_mm(i - 2)
            if i < BPC:
                ph_load(i)
            if 1 <= i <= BPC:
                ph_pool_chunk(i - 1, 0)
            if 2 <= i:
                ph_conv_sig(i - 2)
            if 1 <= i <= BPC:
                b = i - 1
                for c in (1, 2, 3):
                    ph_pool_chunk(b, c)
                ph_mlp(b)
            if 2 <= i:
                ph_final(i - 2, (0, 1))
            if 1 <= i <= BPC:
                b = i - 1
                ph_x2(b)
                ph_chain(b)
                ph_spatial(b)
            if 2 <= i:
                ph_final(i - 2, (2, 3))

    _split_multi_waits(nc)
    return nc


def _get_nc():
    if "nc" not in _cache:
        _cache["nc"] = _build_nc()
    return _cache["nc"]


def _sn_inv(w, u):
    """1/spectral-norm estimate, mirroring reference._sn (f32 numpy)."""
    w = np.asarray(w, np.float32)
    u = np.asarray(u, np.float32)
    Wm = w.reshape(w.shape[0], -1)
    v = u @ Wm
    v = v / max(np.linalg.norm(v), EPS)
    u2 = v @ Wm.T
    u2 = u2 / max(np.linalg.norm(u2), EPS)
    sv = float(np.squeeze(v @ Wm.T @ u2.T))
    return 1.0 / sv


def _prep_in_maps(inputs):
    import ml_dtypes
    bf16 = ml_dtypes.bfloat16
    f = lambda a: np.ascontiguousarray(np.asarray(a, dtype=np.float32))
    x16 = np.ascontiguousarray(
        np.asarray(inputs["x"], dtype=np.float32).astype(bf16))

    W1 = f(inputs["w1"]).reshape(Cr, C) * _sn_inv(inputs["w1"], inputs["u1"])
    W2 = f(inputs["w2"]).reshape(C, Cr) * _sn_inv(inputs["w2"], inputs["u2"])
    taps = (f(inputs["w3"]).reshape(2, 3, 3)
            * _sn_inv(inputs["w3"], inputs["u3"])).astype(np.float32)
    taps = taps.copy()
    taps[0] /= C                               # ca is channel MEAN

    # banded conv stationaries: s6[hin, 3c+dw, hout] = taps[c, hin-hout+1, dw]
    s6 = np.zeros((H, 6, H), np.float32)
    for c in range(2):
        for dw in range(3):
            for hout in range(H):
                for kh in range(3):
                    hin = hout + kh - 1
                    if 0 <= hin < H:
                        s6[hin, 3 * c + dw, hout] = taps[c, kh, dw]
    # cm stationaries for [32,2,66] layout: hin = 2p + r
    s6cm = np.zeros((32, 6, H), np.float32)
    for r in range(2):
        for dw in range(3):
            for p in range(32):
                hin = 2 * p + r
                for hout in range(H):
                    kh = hin - hout + 1
                    if 0 <= kh < 3:
                        s6cm[p, 3 * r + dw, hout] = taps[1, kh, dw]

    # W2^T extended with a 2*b2 bias row (moving hs1[Cr]=1.0)
    w2t = np.ascontiguousarray(W2.T.reshape(Cr, NCH, 128)).astype(np.float32)
    b2row = (2.0 * f(inputs["b2"])).reshape(1, NCH, 128).astype(np.float32)
    w2te = np.ascontiguousarray(np.concatenate([w2t, b2row], axis=0))

    common = {
        "w1t": np.ascontiguousarray(
            W1.T.reshape(NCH, 128, Cr).transpose(1, 0, 2)).astype(np.float32),
        "b1": f(inputs["b1"]).reshape(Cr, 1),
        "w2te": w2te,
        "s6f": s6,
        "s6cm": s6cm,
        "b3r": np.full((H, 1), float(np.asarray(inputs["b3"]).reshape(-1)[0]),
                       np.float32),
        "ones16": np.ones((128, 1), bf16),
        "id16": np.eye(128, dtype=bf16),
        "idf": np.eye(128, dtype=np.float32),
    }
    return [dict(common, x=np.ascontiguousarray(x16[k * BPC:(k + 1) * BPC]))
            for k in range(NCORES)]


def run(inputs, trace=False, **kw):
    nc = _get_nc()
    in_maps = _prep_in_maps(inputs)
    res = run_bass_kernel_spmd(nc, in_maps, list(range(NCORES)), trace=trace, **kw)
    out = np.concatenate(
        [np.asarray(res.results[k]["out"]).astype(np.float32)
         for k in range(NCORES)], axis=0)
    return out, res


def kernel(**inputs) -> np.ndarray:
    out, _ = run(inputs)
    return out


# revision 16
# speedup vs baseline: 1.0779x; 1.0711x over previous
"""CBAM attention (channel + spatial) Trainium2 Bass kernel — bf16 v3.

Full inputs in, full output out. Data-parallel over batch: B=32 samples
split 4-per-core across 8 NeuronCores; params replicated.

v3 design (vs v2 at 227us). Engine budget per sample (ns, measured op
costs): DVE = maxtree 4x2.64u + final 4x2.19u + x2 2x1.08u + chain
3x2.19u + cmaxred 4x1.19u ~= 33u; ACT = avg 4x3.7u + x2 2x3.7u +
smalls ~= 24u; PE ~= 23u. Changes:
  - per-chunk x loads (4 DMAs/sample) with per-chunk pooling chained
    right behind each landing chunk (kills the 21us startup stall).
  - avgpool on ACT accum (scale=1/HWF folded); maxpool per-chunk DVE
    tt-max tree (2x bf16 mode; tensor_scalar-accum / stt / native TTR
    all measured slower or broken in walrus).
  - finals of sample b-1 issued between mlp(b) and x2(b) on DVE,
    filling the s_t(b) MLP wait.
  - csum accumulates into 2-bank PSUM waves DMA'd straight into the
    f32 conv_in (no 1-partition ACT copies).
  - MLP fused: avg/max as 2-col moving per chunk; W2 stationary gets a
    bias row (hs1[64]=1.0) so sigmoid needs no per-chunk bias; one
    [128,4] sigmoid.
  - conv in f32 end to end (s6f f32 stationary, conv_in f32).
  - per-chunk final multiply -> immediate per-chunk store.

Walrus constraints honored: one sync wait per instruction (post-pass
split), no raw-ISA DVE/Pool ops (TTR/mask_reduce/partition_broadcast
all hit "ISA wrong length"), no partition-base mismatch between SBUF
operands.
"""
import numpy as np
from contextlib import ExitStack

import concourse.bass as bass
import concourse.mybir as mybir
from concourse.tile import TileContext
from concourse.bass_utils import run_bass_kernel_spmd

F32 = mybir.dt.float32
BF16 = mybir.dt.bfloat16
ALU = mybir.AluOpType
ACTF = mybir.ActivationFunctionType
AX = mybir.AxisListType

B, C, H, W = 32, 512, 64, 64
NCORES = 8
BPC = B // NCORES          # samples per core
HWF = H * W                # 4096
NCH = C // 128             # 4 channel chunks of 128
Cr = C // 8                # 64
EPS = 1e-12

_cache = {}


def _split_multi_waits(nc):
    import bass_rust
    fn = nc.m.functions[0]
    n_split = 0
    uid = 0
    for bb in list(fn.blocks):
        insts = bb.instructions
        out = []
        changed = False
        for ins in insts:
            si = ins.sync_info
            waits = list(si.on_wait) if si and si.on_wait else []
            if len(waits) > 1:
                changed = True
                for w in waits[:-1]:
                    nop = bass_rust.InstNoOp(name=f"wsplit_{uid}")
                    uid += 1
                    nop.engine = ins.engine
                    nop.sync_info = bass_rust.SyncInfo(on_wait=[w], on_update=[])
                    nc.register_instruction(nop, overwrite=True)
                    out.append(nop)
                    n_split += 1
                ins.sync_info = bass_rust.SyncInfo(
                    on_wait=waits[-1:], on_update=list(si.on_update or []))
            out.append(ins)
        if changed:
            bb.instructions = out
    return n_split


def _build_nc():
    nc = bass.Bass("TRN2", debug=False)

    x_ext = nc.declare_dram_parameter("x", [BPC, C, H, W], BF16, isOutput=False)
    w1t_ext = nc.declare_dram_parameter("w1t", [128, NCH, Cr], F32, isOutput=False)
    b1_ext = nc.declare_dram_parameter("b1", [Cr, 1], F32, isOutput=False)
    w2te_ext = nc.declare_dram_parameter("w2te", [Cr + 1, NCH, 128], F32,
                                         isOutput=False)
    s6f_ext = nc.declare_dram_parameter("s6f", [H, 6, H], F32, isOutput=False)
    s6cm_ext = nc.declare_dram_parameter("s6cm", [32, 6, H], F32, isOutput=False)
    b3r_ext = nc.declare_dram_parameter("b3r", [H, 1], F32, isOutput=False)
    ones16_ext = nc.declare_dram_parameter("ones16", [128, 1], BF16, isOutput=False)
    id16_ext = nc.declare_dram_parameter("id16", [128, 128], BF16, isOutput=False)
    idf_ext = nc.declare_dram_parameter("idf", [128, 128], F32, isOutput=False)
    out_ext = nc.declare_dram_parameter("out", [BPC, C, H, W], BF16, isOutput=True)
    sig_scr = nc.dram_tensor("sig_scratch", [BPC, H, W], BF16)

    # DRAM views: [128 part, chunk, hw]
    x_v = [x_ext[b].rearrange("c h w -> c (h w)").rearrange("(g p) f -> p g f", p=128)
           for b in range(BPC)]
    o_v = [out_ext[b].rearrange("c h w -> c (h w)").rearrange("(g p) f -> p g f", p=128)
           for b in range(BPC)]

    with TileContext(nc) as tc, ExitStack() as ctx:
        const = ctx.enter_context(tc.tile_pool(name="const", bufs=1))
        xpool = ctx.enter_context(tc.tile_pool(name="x", bufs=16))
        t1p = ctx.enter_context(tc.tile_pool(name="t1", bufs=2))
        scr = ctx.enter_context(tc.tile_pool(name="scr", bufs=1))
        sigp = ctx.enter_context(tc.tile_pool(name="sig", bufs=2))
        stats = ctx.enter_context(tc.tile_pool(name="stats", bufs=2))
        convp = ctx.enter_context(tc.tile_pool(name="conv", bufs=2))
        pstp = ctx.enter_context(tc.tile_pool(name="pstp", bufs=2, space="PSUM"))
        pscs = ctx.enter_context(tc.tile_pool(name="pscs", bufs=1, space="PSUM"))
        psmlp = ctx.enter_context(tc.tile_pool(name="psmlp", bufs=1, space="PSUM"))
        psc2 = ctx.enter_context(tc.tile_pool(name="psc2", bufs=1, space="PSUM"))

        # ---------------- load params ----------------
        w1t = const.tile([128, NCH, Cr], F32)
        nc.scalar.dma_start(out=w1t[:], in_=w1t_ext[:])
        b1t = const.tile([Cr, 1], F32)
        nc.scalar.dma_start(out=b1t[:], in_=b1_ext[:])
        w2te = const.tile([Cr + 1, NCH, 128], F32)
        nc.scalar.dma_start(out=w2te[:], in_=w2te_ext[:])
        s6f = const.tile([H, 6, H], F32)
        nc.scalar.dma_start(out=s6f[:], in_=s6f_ext[:])
        s6cm = const.tile([32, 6, H], F32)
        nc.scalar.dma_start(out=s6cm[:], in_=s6cm_ext[:])
        b3rep = const.tile([H, 1], F32)
        nc.scalar.dma_start(out=b3rep[:], in_=b3r_ext[:])
        ones16 = const.tile([128, 1], BF16)
        nc.scalar.dma_start(out=ones16[:], in_=ones16_ext[:])
        ident16 = const.tile([128, 128], BF16)
        nc.scalar.dma_start(out=ident16[:], in_=id16_ext[:])
        identf = const.tile([128, 128], F32)
        nc.scalar.dma_start(out=identf[:], in_=idf_ext[:])

        # shared scratch (write-only / same-engine serial reuse)
        dummy = scr.tile([128, HWF], BF16)        # ACT avg-pool main out
        maxA = scr.tile([128, HWF // 2], BF16)    # maxpool tree ping
        maxB = scr.tile([128, HWF // 4], BF16)    # maxpool tree pong
        chainscr = scr.tile([128, HWF], BF16)     # chain max t23

        st = {}

        def ph_load(b):
            xc = []
            for c in range(NCH):
                t = xpool.tile([128, HWF], BF16, tag="x")
                eng = nc.sync
                if b == 0:
                    # startup-critical: halve for faster first landing
                    eng.dma_start(out=t[:, 0:HWF // 2],
                                  in_=x_v[b][:, c, 0:HWF // 2])
                    eng.dma_start(out=t[:, HWF // 2:HWF],
                                  in_=x_v[b][:, c, HWF // 2:HWF])
                else:
                    eng.dma_start(out=t[:], in_=x_v[b][:, c, :])
                xc.append(t)
            st[b] = {"xc": xc}

        def ph_pool_chunk(b, c):
            z = st[b]
            if c == 0:
                stt8 = stats.tile([128, 2 * NCH], F32, tag="st8")
                z["stt"] = stt8
            stt = z["stt"]
            h2, h4, h8 = HWF // 2, HWF // 4, HWF // 8
            xcc = z["xc"][c]
            # ACT: mean accumulate (scale folds the 1/HW)
            nc.scalar.activation(out=dummy[:], in_=xcc[:], func=ACTF.Copy,
                                 scale=1.0 / HWF,
                                 accum_out=stt[:, 2 * c:2 * c + 1])
            # DVE: max tree
            nc.vector.tensor_tensor(maxA[:, 0:h2], xcc[:, 0:h2],
                                    xcc[:, h2:HWF], ALU.max)
            nc.vector.tensor_tensor(maxB[:, 0:h4], maxA[:, 0:h4],
                                    maxA[:, h4:h2], ALU.max)
            nc.vector.tensor_tensor(maxA[:, 0:h8], maxB[:, 0:h8],
                                    maxB[:, h8:h4], ALU.max)
            nc.vector.tensor_reduce(out=stt[:, 2 * c + 1:2 * c + 2],
                                    in_=maxA[:, 0:h8], axis=AX.X, op=ALU.max)

        def ph_mlp(b):
            z = st[b]
            stt = z["stt"]
            mlpp = psmlp.tile([128, 6], F32, tag="mlp")
            h_ps = mlpp[0:Cr, 0:2]
            for c in range(NCH):
                nc.tensor.matmul(h_ps, w1t[:, c, :], stt[:, 2 * c:2 * c + 2],
                                 start=(c == 0), stop=(c == NCH - 1))
            h_sb = stats.tile([Cr, 2], F32, tag="hsb")
            nc.scalar.activation(out=h_sb[:], in_=h_ps, func=ACTF.Relu,
                                 bias=b1t[:])
            hs1 = stats.tile([Cr + 1, 1], F32, tag="hs1")
            nc.scalar.activation(out=hs1[0:Cr, :], in_=h_sb[:, 0:1],
                                 func=ACTF.Relu, bias=h_sb[:, 1:2])
            nc.gpsimd.memset(hs1[Cr:Cr + 1, :], 1.0)
            a_ps = mlpp[:, 2:6]
            for c in range(NCH):
                nc.tensor.matmul(a_ps[:, c:c + 1], w2te[:, c, :], hs1[:],
                                 start=True, stop=True)
            s_t = stats.tile([128, NCH], F32, tag="sf")
            nc.scalar.activation(out=s_t[:], in_=a_ps, func=ACTF.Sigmoid)
            z["s_t"] = s_t

        def ph_x2(b):
            z = st[b]
            xc, s_t = z["xc"], z["s_t"]
            # DVE chunks 0,1 (ts 4x); ACT chunks 2,3. Last sample all-DVE
            # (shorter tail critical path).
            act_cs = () if b == BPC - 1 else (2, 3)
            for c in range(NCH):
                if c in act_cs:
                    nc.scalar.activation(out=xc[c][:], in_=xc[c][:],
                                         func=ACTF.Copy, scale=s_t[:, c:c + 1])
                else:
                    nc.vector.tensor_scalar(xc[c][:], xc[c][:], s_t[:, c:c + 1],
                                            None, ALU.mult)

        def ph_chain(b):
            z = st[b]
            xc = z["xc"]
            t1 = t1p.tile([128, HWF], BF16, tag="t1")
            nc.vector.tensor_tensor(t1[:], xc[0][:], xc[1][:], ALU.max)
            nc.vector.tensor_tensor(chainscr[:], xc[2][:], xc[3][:], ALU.max)
            for g in range(4):
                lo, hi = 1024 * g, 1024 * (g + 1)
                nc.vector.tensor_tensor(t1[:, lo:hi], t1[:, lo:hi],
                                        chainscr[:, lo:hi], ALU.max)
            z["t1"] = t1

        def ph_spatial(b):
            # csum waves interleaved with cmax transpose groups on PE:
            # wave0, wave1, [tp g0], wave2, [tp g1], wave3, [tp g2], [tp g3]
            z = st[b]
            xc, t1 = z["xc"], z["t1"]
            cia = convp.tile([H, W + 2], F32, tag="cia")
            cim = convp.tile([32, 2, W + 2], F32, tag="cim")
            nc.gpsimd.memset(cia[:, 0:1], 0.0)
            nc.gpsimd.memset(cia[:, W + 1:W + 2], 0.0)
            nc.gpsimd.memset(cim[:, :, 0:1], 0.0)
            nc.gpsimd.memset(cim[:, :, W + 1:W + 2], 0.0)
            cmaxT = stats.tile([128, 32], F32, tag="cmaxT")

            def csum_wave(w):
                ps = pscs.tile([1, 4, 512], F32, tag="cs")
                for c in range(NCH):
                    for j in range(4):
                        q = 4 * w + j
                        nc.tensor.matmul(ps[0:1, j, :], ones16[:],
                                         xc[c][:, 512 * q:512 * (q + 1)],
                                         start=(c == 0), stop=(c == NCH - 1))
                crow = convp.tile([1, 2048], F32, tag="crow")
                nc.scalar.copy(crow[:], ps[:].rearrange("p j w -> p (j w)"))
                nc.scalar.dma_start(
                    out=cia[32 * w:32 * (w + 1), 1:W + 1],
                    in_=crow[:].rearrange("p (h ww) -> p h ww", ww=W))

            def tp_group(g):
                ps = pstp.tile([128, 8, 128], BF16, tag="tp")
                for mm in range(8):
                    m = 8 * g + mm
                    nc.tensor.transpose(ps[:, mm, :], t1[:, 128 * m:128 * (m + 1)],
                                        ident16[:])
                nc.vector.tensor_reduce(out=cmaxT[:, 8 * g:8 * (g + 1)], in_=ps[:],
                                        axis=AX.X, op=ALU.max)

            csum_wave(0)
            tp_group(0)
            csum_wave(1)
            tp_group(1)
            tp_group(2)
            tp_group(3)

            cvt1 = psc2.tile([64, 192], F32, tag="cvt")
            t2ps = cvt1[0:32, 0:128]
            nc.tensor.transpose(t2ps, cmaxT[:], identf[:])
            nc.scalar.copy(cim[:, :, 1:W + 1],
                           t2ps.rearrange("q (r w) -> q r w", r=2))
            z["cia"], z["cim"] = cia, cim

        def ph_conv_mm(b):
            z = st[b]
            cia, cim = z["cia"], z["cim"]
            cvt2 = psc2.tile([64, 192], F32, tag="cvt")
            yps = cvt2[0:H, 128:128 + W]
            j = 0
            for dw in range(3):
                nc.tensor.matmul(yps, s6f[:, dw, :], cia[:, dw:dw + W],
                                 start=(j == 0), stop=False)
                j += 1
            for r in range(2):
                for dw in range(3):
                    nc.tensor.matmul(yps, s6cm[:, 3 * r + dw, :],
                                     cim[:, r, dw:dw + W],
                                     start=False, stop=(j == 8))
                    j += 1
            z["yps"] = yps

        def ph_conv_sig(b):
            z = st[b]
            yps = z["yps"]
            y16 = convp.tile([H, W], BF16, tag="y16")
            nc.scalar.activation(out=y16[:], in_=yps, func=ACTF.Sigmoid,
                                 bias=b3rep[:])
            nc.scalar.dma_start(out=sig_scr[b], in_=y16[:])
            flat = sig_scr[b].rearrange("h w -> (h w)")
            bcast_ap = bass.AP(tensor=flat.tensor, offset=flat.offset,
                               ap=[[0, 128]] + list(flat.ap))
            sigB = sigp.tile([128, HWF], BF16, tag="sigB")
            nc.scalar.dma_start(out=sigB[:], in_=bcast_ap)
            z["sigB"] = sigB

        def ph_final(b, cs):
            z = st[b]
            xc, sigB = z["xc"], z["sigB"]
            for c in cs:
                nc.vector.tensor_tensor(xc[c][:], xc[c][:], sigB[:], ALU.mult)
                nc.sync.dma_start(out=o_v[b][:, c, :], in_=xc[c][:])
            if cs[-1] == NCH - 1:
                del st[b]

        # pipeline: conv_mm(i-2) | load(i) || pool+mlp(i-1) with
        # conv_sig(i-2) after the first avg chunk | final(i-2) |
        # x2+chain+spatial(i-1)
        for i in range(BPC + 2):
            if 2 <= i:
                ph_conv_mm(i - 2)
            if i < BPC:
                ph_load(i)
            if 1 <= i <= BPC:
                ph_pool_chunk(i - 1, 0)
            if 2 <= i:
                ph_conv_sig(i - 2)
            if 1 <= i <= BPC:
                b = i - 1
                for c in (1, 2, 3):
                    ph_pool_chunk(b, c)
                ph_mlp(b)
            if 2 <= i:
                ph_final(i - 2, (0, 1))
            if 1 <= i <= BPC:
                b = i - 1
                ph_x2(b)
                ph_chain(b)
                ph_spatial(b)
            if 2 <= i:
                ph_final(i - 2, (2, 3))

    _split_multi_waits(nc)
    return nc


def _get_nc():
    if "nc" not in _cache:
        _cache["nc"] = _build_nc()
    return _cache["nc"]


def _sn_inv(w, u):
    """1/spectral-norm estimate, mirroring reference._sn (f32 numpy)."""
    w = np.asarray(w, np.float32)
    u = np.asarray(u, np.float32)
    Wm = w.reshape(w.shape[0], -1)
    v = u @ Wm
    v = v / max(np.linalg.norm(v), EPS)
    u2 = v @ Wm.T
    u2 = u2 / max(np.linalg.norm(u2), EPS)
    sv = float(np.squeeze(v @ Wm.T @ u2.T))
    return 1.0 / sv


def _prep_in_maps(inputs):
    import ml_dtypes
    bf16 = ml_dtypes.bfloat16
    f = lambda a: np.ascontiguousarray(np.asarray(a, dtype=np.float32))
    x16 = np.ascontiguousarray(
        np.asarray(inputs["x"], dtype=np.float32).astype(bf16))

    W1 = f(inputs["w1"]).reshape(Cr, C) * _sn_inv(inputs["w1"], inputs["u1"])
    W2 = f(inputs["w2"]).reshape(C, Cr) * _sn_inv(inputs["w2"], inputs["u2"])
    taps = (f(inputs["w3"]).reshape(2, 3, 3)
            * _sn_inv(inputs["w3"], inputs["u3"])).astype(np.float32)
    taps = taps.copy()
    taps[0] /= C                               # ca is channel MEAN

    # banded conv stationaries: s6[hin, 3c+dw, hout] = taps[c, hin-hout+1, dw]
    s6 = np.zeros((H, 6, H), np.float32)
    for c in range(2):
        for dw in range(3):
            for hout in range(H):
                for kh in range(3):
                    hin = hout + kh - 1
                    if 0 <= hin < H:
                        s6[hin, 3 * c + dw, hout] = taps[c, kh, dw]
    # cm stationaries for [32,2,66] layout: hin = 2p + r
    s6cm = np.zeros((32, 6, H), np.float32)
    for r in range(2):
        for dw in range(3):
            for p in range(32):
                hin = 2 * p + r
                for hout in range(H):
                    kh = hin - hout + 1
                    if 0 <= kh < 3:
                        s6cm[p, 3 * r + dw, hout] = taps[1, kh, dw]

    # W2^T extended with a 2*b2 bias row (moving hs1[Cr]=1.0)
    w2t = np.ascontiguousarray(W2.T.reshape(Cr, NCH, 128)).astype(np.float32)
    b2row = (2.0 * f(inputs["b2"])).reshape(1, NCH, 128).astype(np.float32)
    w2te = np.ascontiguousarray(np.concatenate([w2t, b2row], axis=0))

    common = {
        "w1t": np.ascontiguousarray(
            W1.T.reshape(NCH, 128, Cr).transpose(1, 0, 2)).astype(np.float32),
        "b1": f(inputs["b1"]).reshape(Cr, 1),
        "w2te": w2te,
        "s6f": s6,
        "s6cm": s6cm,
        "b3r": np.full((H, 1), float(np.asarray(inputs["b3"]).reshape(-1)[0]),
                       np.float32),
        "ones16": np.ones((128, 1), bf16),
        "id16": np.eye(128, dtype=bf16),
        "idf": np.eye(128, dtype=np.float32),
    }
    return [dict(common, x=np.ascontiguousarray(x16[k * BPC:(k + 1) * BPC]))
            for k in range(NCORES)]


def run(inputs, trace=False, **kw):
    nc = _get_nc()
    in_maps = _prep_in_maps(inputs)
    res = run_bass_kernel_spmd(nc, in_maps, list(range(NCORES)), trace=trace, **kw)
    out = np.concatenate(
        [np.asarray(res.results[k]["out"]).astype(np.float32)
         for k in range(NCORES)], axis=0)
    return out, res


def kernel(**inputs) -> np.ndarray:
    out, _ = run(inputs)
    return out
